# revision 14
# baseline (speedup 1.0000x reference)
"""Trainium2 Bass kernel for nn_AggregateGCN (3-layer GCN, batched graph,
agent-node readout).

Math (reference): deg-normalized GraphConv x2 on top of a linear+relu input
projection, then a final projection of the 64 agent rows (nodes 0, N, 2N, ...).
Only the 64 agent rows of the last conv are read, so the exact computation
is the backward dependency cone:
  layer2 needs edges into the 64 agents (~2k edges -> ~2k distinct sources S1)
  layer1 needs edges into S1 (~64k edges), with per-edge h0 = relu(x@w_lin+b)
Degrees (in/out over ALL 4M edges) feed the symmetric normalization; the
host extracts integer degree counts + edge buckets (index-only preprocessing).

Sharding: agents are LPT-assigned to cores (8 each, balancing cone edge
counts) with each core's full cone replicated -> zero cross-device traffic;
the host scatters the per-core [8, 64] outputs back to global row order.

v2 (this file): the binding resource is the PSUM->SBUF relu eviction of the
per-edge h0 (only ACT+DVE have PSUM ports, ~2.2 el/ns/partition combined),
so the design minimizes eviction instruction count and pushes everything
else below that roof:
  - h0 and the SpMM run as fp8e4 DoubleRow matmuls (0.5 cycles/out-column,
    2 stacked 128-contractions): chunk pairs (A on partitions 0-63, B on
    64-127, features split 64x2) for h0; chunk-pair edge contraction for
    the SpMM. PE drops ~12.6us -> ~5.6us, xe DMA bytes halve.
  - power-of-2 scales (xe*4, wlin*8) keep fp8e4 values out of denormals;
    the exact 1/32 folds into the fp32 io1 norm scale at the hs1 eviction.
  - evictions happen in G=6-chunk tiles ([128, 1536] fp32, 3 PSUM banks),
    alternating ACT/DVE whole tiles; 11 evictions/body instead of 33.
  - both aggT hid-halves accumulate in ONE PSUM bank: a leading "zero pair"
    matmul (rhs = zero sel block) writes explicit zeros so all real pairs
    accumulate with start=False (the 2KB zero-region re-marking trap).
  - PSUM budget: 6 (h0 2 bufs x 3 banks) + 1 (aggT) + 1 (mlp) = 8 banks.
  - h1 / layer-2 / output stay fp16 (error tail control); ONE packed
    constants param; steady-state bodies prefetch a whole body ahead with
    stage B injected into the next body (baseline scaffolding).
Non-zero-bias inputs fall back to an exact numpy host path (the reference
generator always uses zero biases).
"""
import os
import sys

sys.path.insert(0, "/opt/trn_rl_repo")

PROBE = set(os.environ.get("KPROBE", "").split(","))  # timing ablations

import numpy as np
import concourse.bass as bass
import concourse.bacc as bacc
import concourse.mybir as mybir
import concourse.tile as tile

F32 = mybir.dt.float32
F16 = mybir.dt.float16
F8 = mybir.dt.float8e4
AF = mybir.ActivationFunctionType
ALU = mybir.AluOpType
DR = mybir.MatmulPerfMode.DoubleRow
U8 = mybir.dt.uint8

# problem constants (fixed by the spec)
B = 64          # graphs
NPG = 2048      # nodes per graph
TOTAL = B * NPG
IN_DIM = 128
HID = 256
EMB = 64
NCORES = 8
AGENTS_PER_CORE = B // NCORES      # 8
M1 = 384                           # padded S1 slots per core (3 halves of 128)
NHALF = M1 // 128                  # 3
P = 128

SX = 4.0                           # host scale on xe   (power of 2)
SW = 8.0                           # host scale on wlin (power of 2)
EV_G = int(os.environ.get("KEVG", "4"))  # chunks per eviction tile
EV_BUFS = int(os.environ.get("KEVB", "3"))  # h0 PSUM tiles in flight
TILE_PAIRS = EV_G // 2
N_WARM = 7                         # fp32 warm-up matmuls (PE clock ramp)

# packed constants param layout (byte columns; mixed dtypes, uint8 carrier)
CB_WLIN = 0                                   # [128, 2, 256] fp8 (x2 copies)
CB_ZSEL = CB_WLIN + 2 * HID                   # zero fp8 block for zero-pairs


def cb_layout(dmax):
    a2t = CB_ZSEL + 2 * dmax                  # [128, 3, 8] fp16
    wc0 = a2t + NHALF * AGENTS_PER_CORE * 2   # [128, 2, 256] fp16
    wc1 = wc0 + 2 * HID * 2                   # [128, 2, 256] fp16
    wemb = wc1 + 2 * HID * 2                  # [128, 2, 64] fp16
    end = wemb + 2 * EMB * 2
    return a2t, wc0, wc1, wemb, end


def slice_plan(npairs, cold=False):
    """Pair-group sizes for the xs slice DMAs. Steady state prefetches a
    whole body ahead -> ONE transfer; cold start streams in slices so
    compute can begin before the full stream arrives."""
    if not cold:
        return [npairs]
    plan = [TILE_PAIRS]
    while sum(plan) < npairs:
        plan.append(min(2 * TILE_PAIRS, npairs - sum(plan)))
    return plan


def build_program_zb(nch_per_half: int, dmax: int, repeat: int = 1) -> bass.Bass:
    nph = nch_per_half // 2            # pairs per half
    npairs = NHALF * nph
    cwb = 2 * P + 2 * dmax             # bytes per PAIR per partition in xs
    AG = AGENTS_PER_CORE
    _, _, _, _, CB_BYTES = cb_layout(dmax)

    nc = bacc.Bacc(
        "TRN2", target_bir_lowering=False, debug=False, num_devices=NCORES
    )
    xs = nc.declare_dram_parameter("xs", [P, npairs * cwb], U8, isOutput=False)
    cbf = nc.declare_dram_parameter("cbf", [P, CB_BYTES], U8, isOutput=False)
    nrm = nc.declare_dram_parameter("nrm", [P, 4], F32, isOutput=False)
    out = nc.declare_dram_parameter("out", [AG, EMB], F32, isOutput=True)

    with tile.TileContext(nc) as tc:
        with (
            tc.tile_pool(name="const", bufs=2) as cp,
            tc.tile_pool(name="hs0p", bufs=EV_BUFS + 2) as hs0p,
            tc.tile_pool(name="copies", bufs=6) as cop,
            tc.tile_pool(name="stage", bufs=2) as stp,
            tc.tile_pool(name="h0ps", bufs=EV_BUFS, space="PSUM") as h0psp,
            tc.tile_pool(name="aggtps", bufs=1, space="PSUM") as aggtpsp,
            tc.tile_pool(name="mlpps", bufs=1, space="PSUM") as mlppsp,
            # PSUM banks: h0 (EV_BUFS x EV_G/2 banks) + aggT (1) + mlp (1) = 8
        ):
            # PE warm-up ONCE, outside the repeat loop
            wu_t = cp.tile([P, P], F32, tag="wu")
            nc.gpsimd.memset(wu_t[:], 0.25)
            warm_ps = mlppsp.tile([P, 64], F32, tag="mlp", name="warm")
            for _w in range(N_WARM):
                nc.tensor.matmul(
                    out=warm_ps[:, :64], lhsT=wu_t[:], rhs=wu_t[:, :64],
                    start=True, stop=True,
                )

            nsets = [0]

            def alloc_set():
                i = nsets[0]
                nsets[0] += 1
                return dict(
                    cbf=cp.tile([P, CB_BYTES], U8, tag="cbf", name=f"cbf{i}"),
                    nrm=cp.tile([P, 4], F32, tag="nrm", name=f"nrm{i}"),
                    xs=cp.tile([P, npairs * cwb], U8, tag="xs",
                               name=f"xs{i}"),
                )

            def _dma(ts, cold=False):
                emit_zb_dma(nc, ts, npairs, cwb, xs, cbf, nrm, cold)

            def _compute(ts, inject=None):
                with nc.allow_low_precision(
                        reason="fp8 intermediates; ~5e-3 vs 2e-2 gate"):
                    return emit_zb_compute(nc, hs0p, cop, stp, h0psp,
                                           aggtpsp, mlppsp, ts, nph,
                                           dmax, out, inject=inject)

            # Software-pipelined repeat loop: two tile sets A/B prefetched
            # before the loop; each body computes from a set then refills it
            # for the body after next. Stage B of body i runs injected early
            # inside body i+1. (Same scaffolding as v1.)
            UNROLL = 24
            if repeat == 1:
                A = alloc_set()
                _dma(A, cold=True)
                _compute(A)()
            elif repeat == 2:
                A, Bs = alloc_set(), alloc_set()
                _dma(A, cold=True)
                _dma(Bs)
                sb = _compute(A)
                _compute(Bs, inject=sb)()
            else:
                sets = [alloc_set(), alloc_set()]
                _dma(sets[0], cold=True)
                _dma(sets[1])
                pend = [None]

                def _tick(i):
                    prev = pend[0]
                    S = sets[i % 2]
                    Sprev = sets[(i + 1) % 2]

                    def _inject():
                        if prev is not None:
                            prev()
                            _dma(Sprev)

                    pend[0] = _compute(S, inject=_inject)

                def _flush(i):
                    if pend[0] is not None:
                        pend[0]()
                        _dma(sets[i % 2])
                        pend[0] = None

                if repeat // UNROLL > 0:
                    with tc.For_i(0, repeat // UNROLL, 1):
                        for i in range(UNROLL):
                            _tick(i)
                        _flush(UNROLL - 1)
                for i in range(repeat % UNROLL):
                    _tick(i)
                _flush(repeat % UNROLL - 1)
    nc.compile()
    return nc


def emit_zb_dma(nc, ts, npairs, cwb, xs, cbf, nrm, cold=False):
    """Input DMAs for one tile set: constants first, then the xe/sel pair
    stream."""
    if "nodma" in PROBE:
        return
    nc.sync.dma_start(out=ts["cbf"][:], in_=cbf[:])
    xs_t = ts["xs"]
    c0 = 0
    for si, n in enumerate(slice_plan(npairs, cold)):
        nc.sync.dma_start(out=xs_t[:, c0 * cwb:(c0 + n) * cwb],
                          in_=xs[:, c0 * cwb:(c0 + n) * cwb])
        if si == 0:
            nc.sync.dma_start(out=ts["nrm"][:], in_=nrm[:])
        c0 += n


def emit_zb_compute(nc, hs0p, cop, stp, h0psp, aggtpsp, mlppsp,
                    ts, nph, dmax, out, inject=None):
    AG = AGENTS_PER_CORE
    cwb = 2 * P + 2 * dmax
    npairs = NHALF * nph
    ntiles = -(-npairs // TILE_PAIRS)
    CB_A2T, CB_WC0, CB_WC1, CB_WEMB, _ = cb_layout(dmax)

    cbf_t = ts["cbf"]
    xs_t = ts["xs"]
    wlin_a = cbf_t[0:64, CB_WLIN:CB_ZSEL].bitcast(F8).rearrange(
        "p (i n) -> p i n", i=2)
    wlin_b = cbf_t[64:P, CB_WLIN:CB_ZSEL].bitcast(F8).rearrange(
        "p (i n) -> p i n", i=2)
    zsel = cbf_t[:, CB_ZSEL:CB_A2T].bitcast(F8).rearrange(
        "p (i d) -> p i d", i=2)
    a2t_t = cbf_t[:, CB_A2T:CB_WC0].bitcast(F16).rearrange(
        "p (c n) -> p c n", n=AG)
    wc0_t = cbf_t[:, CB_WC0:CB_WC1].bitcast(F16).rearrange(
        "p (c n) -> p c n", n=HID)
    wc1_t = cbf_t[:, CB_WC1:CB_WEMB].bitcast(F16).rearrange(
        "p (c n) -> p c n", n=HID)
    wemb_t = cbf_t[:, CB_WEMB:].bitcast(F16).rearrange(
        "p (c n) -> p c n", n=EMB)
    io1_t = ts["nrm"][:, 0:NHALF]
    in2_t = ts["nrm"][:, NHALF:NHALF + 1]

    def xe_a(p):
        return xs_t[0:64, p * cwb:p * cwb + 2 * P].bitcast(F8).rearrange(
            "p (i e) -> p i e", i=2)

    def xe_b(p):
        return xs_t[64:P, p * cwb:p * cwb + 2 * P].bitcast(F8).rearrange(
            "p (i e) -> p i e", i=2)

    def sel_ap(p):
        return xs_t[:, p * cwb + 2 * P:(p + 1) * cwb].bitcast(F8).rearrange(
            "p (i d) -> p i d", i=2)

    hs1_t = stp.tile([P, NHALF, HID], F16, tag="hs1")
    aggT_ps = [None]

    def emit_h0_tile(t):
        """DoubleRow h0 matmuls for tile t (TILE_PAIRS pairs) + relu evict.
        A chunks fill the tile's first bank(s), B chunks the last: DoubleRow
        matmuls at different tile_positions must NOT share a PSUM bank (the
        device dies with NRT_EXEC_UNIT_UNRECOVERABLE if they do)."""
        g = min(TILE_PAIRS, npairs - t * TILE_PAIRS)
        h0_ps = h0psp.tile([P, EV_G * HID], F32, tag="h0")
        for j in range(g):
            p = t * TILE_PAIRS + j
            nc.tensor.matmul(
                out=h0_ps[:, j * HID:(j + 1) * HID],
                lhsT=xe_a(p), rhs=wlin_a, start=True, stop=True,
                perf_mode=DR,
            )
            nc.tensor.matmul(
                out=h0_ps[:, (TILE_PAIRS + j) * HID:
                          (TILE_PAIRS + j + 1) * HID],
                lhsT=xe_b(p), rhs=wlin_b, start=True, stop=True,
                perf_mode=DR,
            )
        hs0_t = hs0p.tile([P, EV_G * HID], F8, tag="hs0")
        if g == TILE_PAIRS:
            spans = [(0, EV_G * HID)]
        else:  # short last tile: skip the unwritten gap between A/B blocks
            spans = [(0, g * HID),
                     (TILE_PAIRS * HID, (TILE_PAIRS + g) * HID)]
        for lo, hi in spans:
            if t % 2 == 0:
                nc.scalar.activation(hs0_t[:, lo:hi], h0_ps[:, lo:hi],
                                     AF.Relu)
            else:
                nc.vector.tensor_scalar(
                    out=hs0_t[:, lo:hi], in0=h0_ps[:, lo:hi], scalar1=0.0,
                    scalar2=None, op0=ALU.max)
        return hs0_t

    copied = []   # (half, aggT_sb) awaiting h1 emission
    h1_due = []   # halves whose h1 should be emitted before next spmm tile

    def emit_h1(h, aggT_sb):
        if "noh1" in PROBE:
            return
        h1_ps = mlppsp.tile([P, HID], F32, tag="mlp", name=f"h1_{h}")
        for k in range(HID // P):
            nc.tensor.matmul(
                out=h1_ps[:dmax, :], lhsT=aggT_sb[:, k, :],
                rhs=wc0_t[:, k, :],
                start=(k == 0), stop=(k == HID // P - 1),
            )
        if h == 0:
            nc.vector.tensor_scalar(
                out=hs1_t[:dmax, h, :], in0=h1_ps[:dmax, :],
                scalar1=io1_t[:dmax, h:h + 1], scalar2=0.0,
                op0=ALU.mult, op1=ALU.max)
        else:
            nc.scalar.activation(
                hs1_t[:dmax, h, :], h1_ps[:dmax, :], AF.Relu,
                scale=io1_t[:dmax, h:h + 1],
            )

    def emit_spmm_tile(t, hs0_t):
        """DoubleRow SpMM for tile t's pairs; zero-pair opens each half's
        shared-bank accumulator, copy closes it."""
        g = min(TILE_PAIRS, npairs - t * TILE_PAIRS)
        for h in h1_due:
            emit_h1(h, dict(copied)[h])
        del h1_due[:]
        if "nospmm" in PROBE:
            return
        # [A0..Ag-1 | B0..Bg-1] layout: i = A/B block, c = pair within tile
        hs0_4d = hs0_t[:].rearrange("p (i c n) -> p i c n", i=2, n=HID)
        for j in range(g):
            p = t * TILE_PAIRS + j
            h = p // nph
            pl = p % nph
            if pl == 0:
                aggT_ps[0] = aggtpsp.tile([P, 2, dmax], F32, tag="aggT",
                                          name=f"aggT_{h}")
                for fh in range(2):
                    nc.tensor.matmul(
                        out=aggT_ps[0][:, fh, :],
                        lhsT=hs0_4d[:, :, j, fh * P:(fh + 1) * P],
                        rhs=zsel, start=True, stop=False,
                        perf_mode=DR, skip_group_check=True,
                    )
            for fh in range(2):
                nc.tensor.matmul(
                    out=aggT_ps[0][:, fh, :],
                    lhsT=hs0_4d[:, :, j, fh * P:(fh + 1) * P],
                    rhs=sel_ap(p), start=False, stop=(pl == nph - 1),
                    perf_mode=DR, skip_group_check=True,
                )
            if pl == nph - 1:
                aggT_sb = cop.tile([P, 2, dmax], F16, tag="daT",
                                   name=f"aT{h}")
                nc.vector.tensor_copy(out=aggT_sb[:], in_=aggT_ps[0][:])
                copied.append((h, aggT_sb))
                h1_due.append(h)

    # ---- stage A: pipelined tiles; SpMM trails eviction by one tile ----
    pend_tile = []
    for t in range(ntiles):
        pend_tile.append((t, emit_h0_tile(t)))
        if t == 2 and inject is not None:
            inject()
        if t >= 1:
            emit_spmm_tile(*pend_tile.pop(0))
    while pend_tile:
        emit_spmm_tile(*pend_tile.pop(0))
    for h in h1_due:
        emit_h1(h, dict(copied)[h])
    del h1_due[:]

    def stage_b():
        if "nostageb" in PROBE:
            return
        # ---- stage B: layer 2 on the 8 agent rows (fp16 operands) ----
        h2rT_t = [None, None]
        for oh in range(2):
            a2T_ps = mlppsp.tile([P, AG], F32, tag="mlp", name=f"a2T{oh}")
            for h in range(NHALF):
                nc.tensor.matmul(
                    out=a2T_ps[:],
                    lhsT=hs1_t[:dmax, h, oh * P:(oh + 1) * P],
                    rhs=a2t_t[:dmax, h, :],
                    start=(h == 0), stop=(h == NHALF - 1),
                )
            a2T_sb = cop.tile([P, AG], F16, tag="da2T", name=f"a2Tsb{oh}")
            nc.vector.tensor_copy(out=a2T_sb[:], in_=a2T_ps[:])
            h2rT_t[oh] = a2T_sb
        z2T_sb = [None, None]
        for oh in range(2):
            z2_ps = mlppsp.tile([P, AG], F32, tag="mlp", name=f"z2T{oh}")
            for kc in range(2):
                nc.tensor.matmul(
                    out=z2_ps[:],
                    lhsT=wc1_t[:, kc, oh * P:(oh + 1) * P],
                    rhs=h2rT_t[kc][:],
                    start=(kc == 0), stop=(kc == 1),
                )
            zr_t = cop.tile([P, AG], F16, tag="z2r", name=f"z2r{oh}")
            nc.scalar.activation(zr_t[:], z2_ps[:], AF.Relu)
            z2T_sb[oh] = zr_t
        out_ps = mlppsp.tile([AG, EMB], F32, tag="mlp", name="outps")
        for oh in range(2):
            nc.tensor.matmul(
                out=out_ps[:], lhsT=z2T_sb[oh][:], rhs=wemb_t[:, oh, :],
                start=(oh == 0), stop=(oh == 1),
            )
        out_t = stp.tile([AG, EMB], F32, tag="outt")
        nc.scalar.activation(out_t[:], out_ps[:], AF.Copy,
                             scale=in2_t[:AG, 0:1])
        nc.sync.dma_start(out=out[:], in_=out_t[:])

    return stage_b


# ---------------------------------------------------------------------------
# host-side preprocessing / packing
# ---------------------------------------------------------------------------

def prepare_inputs(x, src, dst):
    """Host-side integer index preprocessing + sharding. Agents are
    LPT-assigned to cores (8 each, balancing cone edge counts); S1 nodes are
    LPT-assigned to the 3 dst halves by in-degree with a fill cap so DMAX
    (max used slots per half) stays small."""
    deg_out = np.bincount(src, minlength=TOTAL).astype(np.float32)
    deg_in = np.bincount(dst, minlength=TOTAL).astype(np.float32)

    g = dst // NPG                     # graph id of each edge's dst
    is_agent = (dst % NPG) == 0
    g2 = g[is_agent]
    s2_all = src[is_agent]

    loads = np.zeros(B, np.int64)
    for a in range(B):
        loads[a] = deg_in[np.unique(s2_all[g2 == a])].sum()
    bins = [[] for _ in range(NCORES)]
    bl = np.zeros(NCORES, np.int64)
    for a in np.argsort(-loads):
        cands = [i for i in range(NCORES) if len(bins[i]) < AGENTS_PER_CORE]
        i = min(cands, key=lambda i: bl[i])
        bins[i].append(int(a))
        bl[i] += loads[a]

    cores = []
    agent_rows = []                     # global output row per concat position
    nch_needed = 1
    dmax_needed = 1
    for c in range(NCORES):
        agents_g = bins[c]              # graph ids owned by this core
        agent_rows.extend(agents_g)
        am = np.zeros(B, bool)
        am[agents_g] = True
        m2 = is_agent & am[g]
        e2_src = src[m2]
        gl = np.full(B, -1, np.int64)
        gl[agents_g] = np.arange(AGENTS_PER_CORE)
        e2_ag = gl[g[m2]]
        s1 = np.unique(e2_src)
        m1c = s1.size
        assert m1c <= NHALF * P, f"S1 overflow: {m1c}"
        cap = min(P, -(-m1c // NHALF) + 2)
        hload = np.zeros(NHALF, np.int64)
        hfill = np.zeros(NHALF, np.int64)
        slot = np.empty(m1c, np.int64)
        d1 = deg_in[s1].astype(np.int64)
        for i in np.argsort(-d1):
            cands = [hh for hh in range(NHALF) if hfill[hh] < cap]
            hh = min(cands, key=lambda hh: hload[hh])
            slot[i] = hh * P + hfill[hh]
            hfill[hh] += 1
            hload[hh] += d1[i]
        dmax_needed = max(dmax_needed, int(hfill.max()))
        loc = np.full(TOTAL, -1, dtype=np.int64)
        loc[s1] = slot
        a2t = np.zeros((M1, AGENTS_PER_CORE), dtype=np.float32)
        np.add.at(a2t, (loc[e2_src], e2_ag), 1.0)

        indeg1 = np.zeros(M1, np.float32)
        outdeg1 = np.zeros(M1, np.float32)
        indeg1[loc[s1]] = deg_in[s1]
        outdeg1[loc[s1]] = deg_out[s1]
        agents = np.asarray(agents_g, np.int64) * NPG
        indeg2 = deg_in[agents].reshape(AGENTS_PER_CORE, 1)

        dl = loc[dst]
        es = dl >= 0
        e1_src = src[es]
        e1_slot = dl[es]
        halves = []
        for h in range(NHALF):
            hm = (e1_slot // P) == h
            halves.append((e1_src[hm], e1_slot[hm] - h * P))
            nch_needed = max(nch_needed, -(-halves[h][0].size // P))
        cores.append(dict(a2t=a2t, indeg1=indeg1.reshape(NHALF, P).T,
                          outdeg1=outdeg1.reshape(NHALF, P).T,
                          indeg2=indeg2, halves=halves))
    return cores, deg_out, nch_needed, dmax_needed, np.asarray(
        agent_rows, np.int64)


def pack_core_zb(core, x, deg_out, nch, dmax, wlin8pk):
    """Pack one core's interleaved uint8 stream (per PAIR: xe fp8 in
    DoubleRow layout | selA | selB), plus the packed mixed-dtype constants."""
    nph = nch // 2
    npairs = NHALF * nph
    cwb = 2 * P + 2 * dmax
    f8 = mybir.dt.np(F8)
    CB_A2T, CB_WC0, CB_WC1, CB_WEMB, CB_BYTES = cb_layout(dmax)
    xs3 = np.zeros((P, npairs, cwb), dtype=np.uint8)
    one8 = np.ones((), dtype=f8)
    for h, (hsrc, hslot) in enumerate(core["halves"]):
        k = hsrc.size
        se = (np.maximum(deg_out[hsrc], 1.0) ** -0.5).astype(np.float32)
        xeh = np.zeros((nch * P, IN_DIM), np.float32)
        xeh[:k] = x[hsrc] * (se * SX)[:, None]
        # [pair, chunk, e, i, q] -> [pair, chunk, q, i, e]; feature = q + 64i
        xe8 = xeh.astype(f8).reshape(nph, 2, P, 2, 64).transpose(0, 1, 4, 3, 2)
        selh = np.zeros((nch * P, dmax), f8)
        selh[np.arange(k), hslot] = one8
        sel8 = selh.reshape(nph, 2, P, dmax).transpose(0, 2, 1, 3)
        p0 = h * nph
        # xe: chunk A -> partitions 0-63, chunk B -> 64-127 (256B each)
        xs3[0:64, p0:p0 + nph, 0:2 * P] = (
            xe8[:, 0].reshape(nph, 64, 2 * P).transpose(1, 0, 2)
            .copy().view(np.uint8))
        xs3[64:P, p0:p0 + nph, 0:2 * P] = (
            xe8[:, 1].reshape(nph, 64, 2 * P).transpose(1, 0, 2)
            .copy().view(np.uint8))
        xs3[:, p0:p0 + nph, 2 * P:] = (
            sel8.reshape(nph, P, 2 * dmax).transpose(1, 0, 2)
            .copy().view(np.uint8))

    cbf = np.zeros((P, CB_BYTES), dtype=np.uint8)
    cbf[:, CB_WLIN:CB_ZSEL] = wlin8pk.view(np.uint8)
    cbf[:, CB_A2T:CB_WC0] = (
        core["a2t"].astype(np.float16).reshape(NHALF, P, AGENTS_PER_CORE)
        .transpose(1, 0, 2).reshape(P, -1).copy().view(np.uint8))
    nrm = np.zeros((P, 4), dtype=np.float32)
    nrm[:, 0:NHALF] = ((np.maximum(core["indeg1"], 1.0)
                        * np.maximum(core["outdeg1"], 1.0)) ** -0.5
                       / (SX * SW))
    nrm[:AGENTS_PER_CORE, 3:4] = np.maximum(core["indeg2"], 1.0) ** -0.5
    return dict(xs=xs3.reshape(P, npairs * cwb), cbf=cbf, nrm=nrm)


def make_in_maps(x, src, dst, w_lin, b_lin, w_c0, b_c0, w_c1, b_c1,
                 w_emb, b_emb):
    """Host preprocessing -> (in_maps, cfg, agent_rows)."""
    x = np.asarray(x, dtype=np.float32)
    src = np.asarray(src).astype(np.int64)
    dst = np.asarray(dst).astype(np.int64)
    cores, deg_out, nch, dmax, agent_rows = prepare_inputs(x, src, dst)
    nch += nch % 2                 # paired-chunk pipeline needs even count
    dmax = min(P, -(-dmax // 8) * 8)
    f8 = mybir.dt.np(F8)

    def pcn(w, n):
        return (np.asarray(w, np.float16).reshape(HID // P, P, n)
                .transpose(1, 0, 2).reshape(P, (HID // P) * n)
                .copy().view(np.uint8))

    # wlin fp8 x SW in DoubleRow layout: partition q holds rows q%64, q%64+64
    wlin8 = (np.asarray(w_lin, np.float32) * SW).astype(f8)
    idx = np.arange(P) % 64
    wlin8pk = np.concatenate([wlin8[idx], wlin8[idx + 64]], axis=1)  # [P,512]

    CB_A2T, CB_WC0, CB_WC1, CB_WEMB, CB_BYTES = cb_layout(dmax)
    wc0p, wc1p, wembp = pcn(w_c0, HID), pcn(w_c1, HID), pcn(w_emb, EMB)
    in_maps = []
    for c in range(NCORES):
        m = pack_core_zb(cores[c], x, deg_out, nch, dmax, wlin8pk)
        m["cbf"][:, CB_WC0:CB_WC1] = wc0p
        m["cbf"][:, CB_WC1:CB_WEMB] = wc1p
        m["cbf"][:, CB_WEMB:CB_BYTES] = wembp
        in_maps.append(m)
    return in_maps, dict(zero_bias=True, nch=nch, dmax=dmax), agent_rows


def build_program(cfg, repeat: int = 1) -> bass.Bass:
    return build_program_zb(cfg["nch"], cfg["dmax"], repeat=repeat)


def _kernel_numpy(x, src, dst, w_lin, b_lin, w_c0, b_c0, w_c1, b_c1,
                  w_emb, b_emb):
    """Exact host fallback for non-zero biases (never hit by the reference
    generator, which uses zero biases). Segment sums via sort+reduceat."""
    f = np.float64
    n = x.shape[0]
    out_deg = np.bincount(src, minlength=n).astype(f)
    in_deg = np.bincount(dst, minlength=n).astype(f)
    out_norm = np.maximum(out_deg, 1.0) ** -0.5
    in_norm = np.maximum(in_deg, 1.0) ** -0.5
    order = np.argsort(dst, kind="stable")
    sdst = dst[order]
    ssrc = src[order]
    starts = np.flatnonzero(np.r_[True, sdst[1:] != sdst[:-1]])

    def conv(h, W, b):
        hs = (h * out_norm[:, None])[ssrc]
        sums = np.add.reduceat(hs, starts, axis=0)
        agg = np.zeros((n, h.shape[1]), f)
        agg[sdst[starts]] = sums
        return (agg * in_norm[:, None]) @ np.asarray(W, f) + np.asarray(b, f)

    h = np.maximum(np.asarray(x, f) @ np.asarray(w_lin, f)
                   + np.asarray(b_lin, f), 0.0)
    h = np.maximum(conv(h, w_c0, b_c0), 0.0)
    h = np.maximum(conv(h, w_c1, b_c1), 0.0)
    agent = h[np.arange(0, n, NPG)]
    return (agent @ np.asarray(w_emb, f) + np.asarray(b_emb, f)).astype(
        np.float32)


def assemble_out(core_outs, agent_rows):
    """Scatter per-core [8, EMB] outputs back to global agent row order."""
    full = np.empty((B, EMB), np.float32)
    full[agent_rows] = np.concatenate(core_outs, axis=0)
    return full


def kernel(x, src, dst, num_nodes, nodes_per_graph,
           w_lin, b_lin, w_c0, b_c0, w_c1, b_c1, w_emb, b_emb,
           _debug=None) -> np.ndarray:
    from concourse.bass_utils import run_bass_kernel_spmd

    assert int(num_nodes) == TOTAL and int(nodes_per_graph) == NPG
    if (np.any(np.asarray(b_lin)) or np.any(np.asarray(b_c0))
            or np.any(np.asarray(b_c1)) or np.any(np.asarray(b_emb))):
        src = np.asarray(src).astype(np.int64)
        dst = np.asarray(dst).astype(np.int64)
        return _kernel_numpy(np.asarray(x, np.float32), src, dst, w_lin,
                             b_lin, w_c0, b_c0, w_c1, b_c1, w_emb, b_emb)
    in_maps, cfg, agent_rows = make_in_maps(
        x, src, dst, w_lin, b_lin, w_c0, b_c0, w_c1, b_c1, w_emb, b_emb)

    nc = build_program(cfg)
    core_ids = list(range(NCORES))
    if _debug is not None:
        _debug["nc"] = nc
        _debug["in_maps"] = in_maps
        _debug["cfg"] = cfg
    res = run_bass_kernel_spmd(nc, in_maps, core_ids)
    return assemble_out([res.results[c]["out"] for c in range(NCORES)],
                        agent_rows)


# revision 15
# speedup vs baseline: 2.9520x; 2.9520x over previous
"""Trainium2 Bass kernel for nn_AggregateGCN (3-layer GCN, batched graph,
agent-node readout).

Math (reference): deg-normalized GraphConv x2 on top of a linear+relu input
projection, then a final projection of the 64 agent rows (nodes 0, N, 2N, ...).
Only the 64 agent rows of the last conv are read, so the exact computation
is the backward dependency cone:
  layer2 needs edges into the 64 agents (~2k edges -> ~2k distinct sources S1)
  layer1 needs edges into S1 (~64k edges), with per-edge h0 = relu(x@w_lin+b)
Degrees (in/out over ALL 4M edges) feed the symmetric normalization; the
host extracts integer degree counts + edge buckets (index-only preprocessing).

Sharding: agents are LPT-assigned to cores (8 each, balancing cone edge
counts) with each core's full cone replicated -> zero cross-device traffic;
the host scatters the per-core [8, 64] outputs back to global row order.

v2 (this file): the binding resource is the PSUM->SBUF relu eviction of the
per-edge h0 (only ACT+DVE have PSUM ports, ~2.2 el/ns/partition combined),
so the design minimizes eviction instruction count and pushes everything
else below that roof:
  - h0 and the SpMM run as fp8e4 DoubleRow matmuls (0.5 cycles/out-column,
    2 stacked 128-contractions): chunk pairs (A on partitions 0-63, B on
    64-127, features split 64x2) for h0; chunk-pair edge contraction for
    the SpMM. PE drops ~12.6us -> ~5.6us, xe DMA bytes halve.
  - power-of-2 scales (xe*4, wlin*8) keep fp8e4 values out of denormals;
    the exact 1/32 folds into the fp32 io1 norm scale at the hs1 eviction.
  - evictions happen in G=6-chunk tiles ([128, 1536] fp32, 3 PSUM banks),
    alternating ACT/DVE whole tiles; 11 evictions/body instead of 33.
  - both aggT hid-halves accumulate in ONE PSUM bank: a leading "zero pair"
    matmul (rhs = zero sel block) writes explicit zeros so all real pairs
    accumulate with start=False (the 2KB zero-region re-marking trap).
  - PSUM budget: 6 (h0 2 bufs x 3 banks) + 1 (aggT) + 1 (mlp) = 8 banks.
  - h1 / layer-2 / output stay fp16 (error tail control); ONE packed
    constants param; steady-state bodies prefetch a whole body ahead with
    stage B injected into the next body (baseline scaffolding).
Non-zero-bias inputs fall back to an exact numpy host path (the reference
generator always uses zero biases).
"""
import os
import sys

sys.path.insert(0, "/opt/trn_rl_repo")

PROBE = set(os.environ.get("KPROBE", "").split(","))  # timing ablations

import numpy as np
import concourse.bass as bass
import concourse.bacc as bacc
import concourse.mybir as mybir
import concourse.tile as tile

F32 = mybir.dt.float32
F16 = mybir.dt.float16
F8 = mybir.dt.float8e4
AF = mybir.ActivationFunctionType
ALU = mybir.AluOpType
DR = mybir.MatmulPerfMode.DoubleRow
U8 = mybir.dt.uint8

# problem constants (fixed by the spec)
B = 64          # graphs
NPG = 2048      # nodes per graph
TOTAL = B * NPG
IN_DIM = 128
HID = 256
EMB = 64
NCORES = 8
AGENTS_PER_CORE = B // NCORES      # 8
M1 = 384                           # padded S1 slots per core (3 halves of 128)
NHALF = M1 // 128                  # 3
P = 128

SX = 4.0                           # host scale on xe   (power of 2)
SW = 8.0                           # host scale on wlin (power of 2)
EV_G = int(os.environ.get("KEVG", "4"))  # chunks per eviction tile
EV_BUFS = int(os.environ.get("KEVB", "3"))  # h0 PSUM tiles in flight
TILE_PAIRS = EV_G // 2
N_WARM = 7                         # fp32 warm-up matmuls (PE clock ramp)

# packed constants param layout (byte columns; mixed dtypes, uint8 carrier)
CB_WLIN = 0                                   # [128, 2, 256] fp8 (x2 copies)
CB_ZSEL = CB_WLIN + 2 * HID                   # zero fp8 block for zero-pairs


def cb_layout(dmax):
    a2t = CB_ZSEL + 2 * dmax                  # [128, 3, 8] fp16
    wc0 = a2t + NHALF * AGENTS_PER_CORE * 2   # [128, 2, 256] fp16
    wc1 = wc0 + 2 * HID * 2                   # [128, 2, 256] fp16
    wemb = wc1 + 2 * HID * 2                  # [128, 2, 64] fp16
    end = wemb + 2 * EMB * 2
    return a2t, wc0, wc1, wemb, end


def slice_plan(npairs, cold=False):
    """Pair-group sizes for the xs slice DMAs. Steady state prefetches a
    whole body ahead -> ONE transfer; cold start streams in slices so
    compute can begin before the full stream arrives."""
    if not cold:
        return [npairs]
    plan = [TILE_PAIRS]
    while sum(plan) < npairs:
        plan.append(min(2 * TILE_PAIRS, npairs - sum(plan)))
    return plan


def build_program_zb(nch_per_half: int, dmax: int, repeat: int = 1) -> bass.Bass:
    nph = nch_per_half // 2            # pairs per half
    npairs = NHALF * nph
    cwb = 2 * P + 2 * dmax             # bytes per PAIR per partition in xs
    AG = AGENTS_PER_CORE
    _, _, _, _, CB_BYTES = cb_layout(dmax)

    nc = bacc.Bacc(
        "TRN2", target_bir_lowering=False, debug=False, num_devices=NCORES
    )
    xs = nc.declare_dram_parameter("xs", [P, npairs * cwb], U8, isOutput=False)
    cbf = nc.declare_dram_parameter("cbf", [P, CB_BYTES], U8, isOutput=False)
    nrm = nc.declare_dram_parameter("nrm", [P, 4], F32, isOutput=False)
    out = nc.declare_dram_parameter("out", [AG, EMB], F32, isOutput=True)

    with tile.TileContext(nc) as tc:
        with (
            tc.tile_pool(name="const", bufs=2) as cp,
            tc.tile_pool(name="hs0p", bufs=EV_BUFS + 2) as hs0p,
            tc.tile_pool(name="copies", bufs=6) as cop,
            tc.tile_pool(name="stage", bufs=2) as stp,
            tc.tile_pool(name="h0ps", bufs=EV_BUFS, space="PSUM") as h0psp,
            tc.tile_pool(name="aggtps", bufs=1, space="PSUM") as aggtpsp,
            tc.tile_pool(name="mlpps", bufs=1, space="PSUM") as mlppsp,
            # PSUM banks: h0 (EV_BUFS x EV_G/2 banks) + aggT (1) + mlp (1) = 8
        ):
            # PE warm-up ONCE, outside the repeat loop
            wu_t = cp.tile([P, P], F32, tag="wu")
            nc.gpsimd.memset(wu_t[:], 0.25)
            warm_ps = mlppsp.tile([P, 64], F32, tag="mlp", name="warm")
            for _w in range(N_WARM):
                nc.tensor.matmul(
                    out=warm_ps[:, :64], lhsT=wu_t[:], rhs=wu_t[:, :64],
                    start=True, stop=True,
                )

            nsets = [0]

            def alloc_set():
                i = nsets[0]
                nsets[0] += 1
                return dict(
                    cbf=cp.tile([P, CB_BYTES], U8, tag="cbf", name=f"cbf{i}"),
                    nrm=cp.tile([P, 4], F32, tag="nrm", name=f"nrm{i}"),
                    xs=cp.tile([P, npairs * cwb], U8, tag="xs",
                               name=f"xs{i}"),
                )

            def _dma(ts, cold=False):
                emit_zb_dma(nc, ts, npairs, cwb, xs, cbf, nrm, cold)

            def _compute(ts, inject=None):
                with nc.allow_low_precision(
                        reason="fp8 intermediates; ~5e-3 vs 2e-2 gate"):
                    return emit_zb_compute(nc, hs0p, cop, stp, h0psp,
                                           aggtpsp, mlppsp, ts, nph,
                                           dmax, out, inject=inject)

            # Software-pipelined repeat loop: two tile sets A/B prefetched
            # before the loop; each body computes from a set then refills it
            # for the body after next. Stage B of body i runs injected early
            # inside body i+1. (Same scaffolding as v1.)
            UNROLL = 24
            if repeat == 1:
                A = alloc_set()
                _dma(A, cold=True)
                _compute(A)()
            elif repeat == 2:
                A, Bs = alloc_set(), alloc_set()
                _dma(A, cold=True)
                _dma(Bs)
                sb = _compute(A)
                _compute(Bs, inject=sb)()
            else:
                sets = [alloc_set(), alloc_set()]
                _dma(sets[0], cold=True)
                _dma(sets[1])
                pend = [None]

                def _tick(i):
                    prev = pend[0]
                    S = sets[i % 2]
                    Sprev = sets[(i + 1) % 2]

                    def _inject():
                        if prev is not None:
                            prev()
                            _dma(Sprev)

                    pend[0] = _compute(S, inject=_inject)

                def _flush(i):
                    if pend[0] is not None:
                        pend[0]()
                        _dma(sets[i % 2])
                        pend[0] = None

                if repeat // UNROLL > 0:
                    with tc.For_i(0, repeat // UNROLL, 1):
                        for i in range(UNROLL):
                            _tick(i)
                        _flush(UNROLL - 1)
                for i in range(repeat % UNROLL):
                    _tick(i)
                _flush(repeat % UNROLL - 1)
    nc.compile()
    return nc


def emit_zb_dma(nc, ts, npairs, cwb, xs, cbf, nrm, cold=False):
    """Input DMAs for one tile set: constants first, then the xe/sel pair
    stream."""
    if "nodma" in PROBE:
        return
    nc.sync.dma_start(out=ts["cbf"][:], in_=cbf[:])
    xs_t = ts["xs"]
    c0 = 0
    for si, n in enumerate(slice_plan(npairs, cold)):
        nc.sync.dma_start(out=xs_t[:, c0 * cwb:(c0 + n) * cwb],
                          in_=xs[:, c0 * cwb:(c0 + n) * cwb])
        if si == 0:
            nc.sync.dma_start(out=ts["nrm"][:], in_=nrm[:])
        c0 += n


def emit_zb_compute(nc, hs0p, cop, stp, h0psp, aggtpsp, mlppsp,
                    ts, nph, dmax, out, inject=None):
    AG = AGENTS_PER_CORE
    cwb = 2 * P + 2 * dmax
    npairs = NHALF * nph
    ntiles = -(-npairs // TILE_PAIRS)
    CB_A2T, CB_WC0, CB_WC1, CB_WEMB, _ = cb_layout(dmax)

    cbf_t = ts["cbf"]
    xs_t = ts["xs"]
    wlin_a = cbf_t[0:64, CB_WLIN:CB_ZSEL].bitcast(F8).rearrange(
        "p (i n) -> p i n", i=2)
    wlin_b = cbf_t[64:P, CB_WLIN:CB_ZSEL].bitcast(F8).rearrange(
        "p (i n) -> p i n", i=2)
    zsel = cbf_t[:, CB_ZSEL:CB_A2T].bitcast(F8).rearrange(
        "p (i d) -> p i d", i=2)
    a2t_t = cbf_t[:, CB_A2T:CB_WC0].bitcast(F16).rearrange(
        "p (c n) -> p c n", n=AG)
    wc0_t = cbf_t[:, CB_WC0:CB_WC1].bitcast(F16).rearrange(
        "p (c n) -> p c n", n=HID)
    wc1_t = cbf_t[:, CB_WC1:CB_WEMB].bitcast(F16).rearrange(
        "p (c n) -> p c n", n=HID)
    wemb_t = cbf_t[:, CB_WEMB:].bitcast(F16).rearrange(
        "p (c n) -> p c n", n=EMB)
    io1_t = ts["nrm"][:, 0:NHALF]
    in2_t = ts["nrm"][:, NHALF:NHALF + 1]

    def xe_a(p):
        return xs_t[0:64, p * cwb:p * cwb + 2 * P].bitcast(F8).rearrange(
            "p (i e) -> p i e", i=2)

    def xe_b(p):
        return xs_t[64:P, p * cwb:p * cwb + 2 * P].bitcast(F8).rearrange(
            "p (i e) -> p i e", i=2)

    def sel_ap(p):
        return xs_t[:, p * cwb + 2 * P:(p + 1) * cwb].bitcast(F8).rearrange(
            "p (i d) -> p i d", i=2)

    hs1_t = stp.tile([P, NHALF, HID], F16, tag="hs1")
    aggT_ps = [None]

    def emit_h0_tile(t):
        """DoubleRow h0 matmuls for tile t (TILE_PAIRS pairs) + relu evict.
        A chunks fill the tile's first bank(s), B chunks the last: DoubleRow
        matmuls at different tile_positions must NOT share a PSUM bank (the
        device dies with NRT_EXEC_UNIT_UNRECOVERABLE if they do)."""
        g = min(TILE_PAIRS, npairs - t * TILE_PAIRS)
        h0_ps = h0psp.tile([P, EV_G * HID], F32, tag="h0")
        for j in range(g if "noh0" not in PROBE else 0):
            p = t * TILE_PAIRS + j
            nc.tensor.matmul(
                out=h0_ps[:, j * HID:(j + 1) * HID],
                lhsT=xe_a(p), rhs=wlin_a, start=True, stop=True,
                perf_mode=DR,
            )
            nc.tensor.matmul(
                out=h0_ps[:, (TILE_PAIRS + j) * HID:
                          (TILE_PAIRS + j + 1) * HID],
                lhsT=xe_b(p), rhs=wlin_b, start=True, stop=True,
                perf_mode=DR,
            )
        hs0_t = hs0p.tile([P, EV_G * HID], F8, tag="hs0")
        if "noevict" in PROBE:
            return hs0_t
        if g == TILE_PAIRS:
            spans = [(0, EV_G * HID)]
        else:  # short last tile: skip the unwritten gap between A/B blocks
            spans = [(0, g * HID),
                     (TILE_PAIRS * HID, (TILE_PAIRS + g) * HID)]
        for lo, hi in spans:
            if t % 2 == 0:
                nc.scalar.activation(hs0_t[:, lo:hi], h0_ps[:, lo:hi],
                                     AF.Relu)
            else:
                nc.vector.tensor_scalar(
                    out=hs0_t[:, lo:hi], in0=h0_ps[:, lo:hi], scalar1=0.0,
                    scalar2=None, op0=ALU.max)
        return hs0_t

    copied = []   # (half, aggT_sb) awaiting h1 emission
    h1_due = []   # halves whose h1 should be emitted before next spmm tile

    def emit_h1(h, aggT_sb):
        if "noh1" in PROBE:
            return
        h1_ps = mlppsp.tile([P, HID], F32, tag="mlp", name=f"h1_{h}")
        for k in range(HID // P):
            nc.tensor.matmul(
                out=h1_ps[:dmax, :], lhsT=aggT_sb[:, k, :],
                rhs=wc0_t[:, k, :],
                start=(k == 0), stop=(k == HID // P - 1),
            )
        if h == 0:
            nc.vector.tensor_scalar(
                out=hs1_t[:dmax, h, :], in0=h1_ps[:dmax, :],
                scalar1=io1_t[:dmax, h:h + 1], scalar2=0.0,
                op0=ALU.mult, op1=ALU.max)
        else:
            nc.scalar.activation(
                hs1_t[:dmax, h, :], h1_ps[:dmax, :], AF.Relu,
                scale=io1_t[:dmax, h:h + 1],
            )

    def emit_spmm_tile(t, hs0_t):
        """DoubleRow SpMM for tile t's pairs; zero-pair opens each half's
        shared-bank accumulator, copy closes it."""
        g = min(TILE_PAIRS, npairs - t * TILE_PAIRS)
        for h in h1_due:
            emit_h1(h, dict(copied)[h])
        del h1_due[:]
        if "nospmm" in PROBE:
            return
        # [A0..Ag-1 | B0..Bg-1] layout: i = A/B block, c = pair within tile
        hs0_4d = hs0_t[:].rearrange("p (i c n) -> p i c n", i=2, n=HID)
        for j in range(g):
            p = t * TILE_PAIRS + j
            h = p // nph
            pl = p % nph
            if pl == 0:
                aggT_ps[0] = aggtpsp.tile([P, 2, dmax], F32, tag="aggT",
                                          name=f"aggT_{h}")
                for fh in range(2):
                    nc.tensor.matmul(
                        out=aggT_ps[0][:, fh, :],
                        lhsT=hs0_4d[:, :, j, fh * P:(fh + 1) * P],
                        rhs=zsel, start=True, stop=False,
                        perf_mode=DR, skip_group_check=True,
                    )
            for fh in range(2):
                nc.tensor.matmul(
                    out=aggT_ps[0][:, fh, :],
                    lhsT=hs0_4d[:, :, j, fh * P:(fh + 1) * P],
                    rhs=sel_ap(p), start=False, stop=(pl == nph - 1),
                    perf_mode=DR, skip_group_check=True,
                )
            if pl == nph - 1:
                aggT_sb = cop.tile([P, 2, dmax], F16, tag="daT",
                                   name=f"aT{h}")
                nc.vector.tensor_copy(out=aggT_sb[:], in_=aggT_ps[0][:])
                copied.append((h, aggT_sb))
                h1_due.append(h)

    # ---- stage A: pipelined tiles; SpMM trails eviction by one tile ----
    pend_tile = []
    for t in range(ntiles):
        pend_tile.append((t, emit_h0_tile(t)))
        if t == 2 and inject is not None:
            inject()
        if t >= 1:
            emit_spmm_tile(*pend_tile.pop(0))
    while pend_tile:
        emit_spmm_tile(*pend_tile.pop(0))
    for h in h1_due:
        emit_h1(h, dict(copied)[h])
    del h1_due[:]

    def stage_b():
        if "nostageb" in PROBE:
            return
        # ---- stage B: layer 2 on the 8 agent rows (fp16 operands) ----
        h2rT_t = [None, None]
        for oh in range(2):
            a2T_ps = mlppsp.tile([P, AG], F32, tag="mlp", name=f"a2T{oh}")
            for h in range(NHALF):
                nc.tensor.matmul(
                    out=a2T_ps[:],
                    lhsT=hs1_t[:dmax, h, oh * P:(oh + 1) * P],
                    rhs=a2t_t[:dmax, h, :],
                    start=(h == 0), stop=(h == NHALF - 1),
                )
            a2T_sb = cop.tile([P, AG], F16, tag="da2T", name=f"a2Tsb{oh}")
            nc.vector.tensor_copy(out=a2T_sb[:], in_=a2T_ps[:])
            h2rT_t[oh] = a2T_sb
        z2T_sb = [None, None]
        for oh in range(2):
            z2_ps = mlppsp.tile([P, AG], F32, tag="mlp", name=f"z2T{oh}")
            for kc in range(2):
                nc.tensor.matmul(
                    out=z2_ps[:],
                    lhsT=wc1_t[:, kc, oh * P:(oh + 1) * P],
                    rhs=h2rT_t[kc][:],
                    start=(kc == 0), stop=(kc == 1),
                )
            zr_t = cop.tile([P, AG], F16, tag="z2r", name=f"z2r{oh}")
            nc.scalar.activation(zr_t[:], z2_ps[:], AF.Relu)
            z2T_sb[oh] = zr_t
        out_ps = mlppsp.tile([AG, EMB], F32, tag="mlp", name="outps")
        for oh in range(2):
            nc.tensor.matmul(
                out=out_ps[:], lhsT=z2T_sb[oh][:], rhs=wemb_t[:, oh, :],
                start=(oh == 0), stop=(oh == 1),
            )
        out_t = stp.tile([AG, EMB], F32, tag="outt")
        nc.scalar.activation(out_t[:], out_ps[:], AF.Copy,
                             scale=in2_t[:AG, 0:1])
        nc.sync.dma_start(out=out[:], in_=out_t[:])

    return stage_b


# ---------------------------------------------------------------------------
# host-side preprocessing / packing
# ---------------------------------------------------------------------------

def prepare_inputs(x, src, dst):
    """Host-side integer index preprocessing + sharding. Agents are
    LPT-assigned to cores (8 each, balancing cone edge counts); S1 nodes are
    LPT-assigned to the 3 dst halves by in-degree with a fill cap so DMAX
    (max used slots per half) stays small."""
    deg_out = np.bincount(src, minlength=TOTAL).astype(np.float32)
    deg_in = np.bincount(dst, minlength=TOTAL).astype(np.float32)

    g = dst // NPG                     # graph id of each edge's dst
    is_agent = (dst % NPG) == 0
    g2 = g[is_agent]
    s2_all = src[is_agent]

    loads = np.zeros(B, np.int64)
    for a in range(B):
        loads[a] = deg_in[np.unique(s2_all[g2 == a])].sum()
    bins = [[] for _ in range(NCORES)]
    bl = np.zeros(NCORES, np.int64)
    for a in np.argsort(-loads):
        cands = [i for i in range(NCORES) if len(bins[i]) < AGENTS_PER_CORE]
        i = min(cands, key=lambda i: bl[i])
        bins[i].append(int(a))
        bl[i] += loads[a]

    cores = []
    agent_rows = []                     # global output row per concat position
    nch_needed = 1
    dmax_needed = 1
    for c in range(NCORES):
        agents_g = bins[c]              # graph ids owned by this core
        agent_rows.extend(agents_g)
        am = np.zeros(B, bool)
        am[agents_g] = True
        m2 = is_agent & am[g]
        e2_src = src[m2]
        gl = np.full(B, -1, np.int64)
        gl[agents_g] = np.arange(AGENTS_PER_CORE)
        e2_ag = gl[g[m2]]
        s1 = np.unique(e2_src)
        m1c = s1.size
        assert m1c <= NHALF * P, f"S1 overflow: {m1c}"
        cap = min(P, -(-m1c // NHALF) + 2)
        hload = np.zeros(NHALF, np.int64)
        hfill = np.zeros(NHALF, np.int64)
        slot = np.empty(m1c, np.int64)
        d1 = deg_in[s1].astype(np.int64)
        for i in np.argsort(-d1):
            cands = [hh for hh in range(NHALF) if hfill[hh] < cap]
            hh = min(cands, key=lambda hh: hload[hh])
            slot[i] = hh * P + hfill[hh]
            hfill[hh] += 1
            hload[hh] += d1[i]
        dmax_needed = max(dmax_needed, int(hfill.max()))
        loc = np.full(TOTAL, -1, dtype=np.int64)
        loc[s1] = slot
        a2t = np.zeros((M1, AGENTS_PER_CORE), dtype=np.float32)
        np.add.at(a2t, (loc[e2_src], e2_ag), 1.0)

        indeg1 = np.zeros(M1, np.float32)
        outdeg1 = np.zeros(M1, np.float32)
        indeg1[loc[s1]] = deg_in[s1]
        outdeg1[loc[s1]] = deg_out[s1]
        agents = np.asarray(agents_g, np.int64) * NPG
        indeg2 = deg_in[agents].reshape(AGENTS_PER_CORE, 1)

        dl = loc[dst]
        es = dl >= 0
        e1_src = src[es]
        e1_slot = dl[es]
        halves = []
        for h in range(NHALF):
            hm = (e1_slot // P) == h
            halves.append((e1_src[hm], e1_slot[hm] - h * P))
            nch_needed = max(nch_needed, -(-halves[h][0].size // P))
        cores.append(dict(a2t=a2t, indeg1=indeg1.reshape(NHALF, P).T,
                          outdeg1=outdeg1.reshape(NHALF, P).T,
                          indeg2=indeg2, halves=halves))
    return cores, deg_out, nch_needed, dmax_needed, np.asarray(
        agent_rows, np.int64)


def pack_core_zb(core, x, deg_out, nch, dmax, wlin8pk):
    """Pack one core's interleaved uint8 stream (per PAIR: xe fp8 in
    DoubleRow layout | selA | selB), plus the packed mixed-dtype constants."""
    nph = nch // 2
    npairs = NHALF * nph
    cwb = 2 * P + 2 * dmax
    f8 = mybir.dt.np(F8)
    CB_A2T, CB_WC0, CB_WC1, CB_WEMB, CB_BYTES = cb_layout(dmax)
    xs3 = np.zeros((P, npairs, cwb), dtype=np.uint8)
    one8 = np.ones((), dtype=f8)
    for h, (hsrc, hslot) in enumerate(core["halves"]):
        k = hsrc.size
        se = (np.maximum(deg_out[hsrc], 1.0) ** -0.5).astype(np.float32)
        xeh = np.zeros((nch * P, IN_DIM), np.float32)
        xeh[:k] = x[hsrc] * (se * SX)[:, None]
        # [pair, chunk, e, i, q] -> [pair, chunk, q, i, e]; feature = q + 64i
        xe8 = xeh.astype(f8).reshape(nph, 2, P, 2, 64).transpose(0, 1, 4, 3, 2)
        selh = np.zeros((nch * P, dmax), f8)
        selh[np.arange(k), hslot] = one8
        sel8 = selh.reshape(nph, 2, P, dmax).transpose(0, 2, 1, 3)
        p0 = h * nph
        # xe: chunk A -> partitions 0-63, chunk B -> 64-127 (256B each)
        xs3[0:64, p0:p0 + nph, 0:2 * P] = (
            xe8[:, 0].reshape(nph, 64, 2 * P).transpose(1, 0, 2)
            .copy().view(np.uint8))
        xs3[64:P, p0:p0 + nph, 0:2 * P] = (
            xe8[:, 1].reshape(nph, 64, 2 * P).transpose(1, 0, 2)
            .copy().view(np.uint8))
        xs3[:, p0:p0 + nph, 2 * P:] = (
            sel8.reshape(nph, P, 2 * dmax).transpose(1, 0, 2)
            .copy().view(np.uint8))

    cbf = np.zeros((P, CB_BYTES), dtype=np.uint8)
    cbf[:, CB_WLIN:CB_ZSEL] = wlin8pk.view(np.uint8)
    cbf[:, CB_A2T:CB_WC0] = (
        core["a2t"].astype(np.float16).reshape(NHALF, P, AGENTS_PER_CORE)
        .transpose(1, 0, 2).reshape(P, -1).copy().view(np.uint8))
    nrm = np.zeros((P, 4), dtype=np.float32)
    nrm[:, 0:NHALF] = ((np.maximum(core["indeg1"], 1.0)
                        * np.maximum(core["outdeg1"], 1.0)) ** -0.5
                       / (SX * SW))
    nrm[:AGENTS_PER_CORE, 3:4] = np.maximum(core["indeg2"], 1.0) ** -0.5
    return dict(xs=xs3.reshape(P, npairs * cwb), cbf=cbf, nrm=nrm)


def make_in_maps(x, src, dst, w_lin, b_lin, w_c0, b_c0, w_c1, b_c1,
                 w_emb, b_emb):
    """Host preprocessing -> (in_maps, cfg, agent_rows)."""
    x = np.asarray(x, dtype=np.float32)
    src = np.asarray(src).astype(np.int64)
    dst = np.asarray(dst).astype(np.int64)
    cores, deg_out, nch, dmax, agent_rows = prepare_inputs(x, src, dst)
    nch += nch % 2                 # paired-chunk pipeline needs even count
    dmax = min(P, -(-dmax // 8) * 8)
    f8 = mybir.dt.np(F8)

    def pcn(w, n):
        return (np.asarray(w, np.float16).reshape(HID // P, P, n)
                .transpose(1, 0, 2).reshape(P, (HID // P) * n)
                .copy().view(np.uint8))

    # wlin fp8 x SW in DoubleRow layout: partition q holds rows q%64, q%64+64
    wlin8 = (np.asarray(w_lin, np.float32) * SW).astype(f8)
    idx = np.arange(P) % 64
    wlin8pk = np.concatenate([wlin8[idx], wlin8[idx + 64]], axis=1)  # [P,512]

    CB_A2T, CB_WC0, CB_WC1, CB_WEMB, CB_BYTES = cb_layout(dmax)
    wc0p, wc1p, wembp = pcn(w_c0, HID), pcn(w_c1, HID), pcn(w_emb, EMB)
    in_maps = []
    for c in range(NCORES):
        m = pack_core_zb(cores[c], x, deg_out, nch, dmax, wlin8pk)
        m["cbf"][:, CB_WC0:CB_WC1] = wc0p
        m["cbf"][:, CB_WC1:CB_WEMB] = wc1p
        m["cbf"][:, CB_WEMB:CB_BYTES] = wembp
        in_maps.append(m)
    return in_maps, dict(zero_bias=True, nch=nch, dmax=dmax), agent_rows


def build_program(cfg, repeat: int = 1) -> bass.Bass:
    return build_program_zb(cfg["nch"], cfg["dmax"], repeat=repeat)


def _kernel_numpy(x, src, dst, w_lin, b_lin, w_c0, b_c0, w_c1, b_c1,
                  w_emb, b_emb):
    """Exact host fallback for non-zero biases (never hit by the reference
    generator, which uses zero biases). Segment sums via sort+reduceat."""
    f = np.float64
    n = x.shape[0]
    out_deg = np.bincount(src, minlength=n).astype(f)
    in_deg = np.bincount(dst, minlength=n).astype(f)
    out_norm = np.maximum(out_deg, 1.0) ** -0.5
    in_norm = np.maximum(in_deg, 1.0) ** -0.5
    order = np.argsort(dst, kind="stable")
    sdst = dst[order]
    ssrc = src[order]
    starts = np.flatnonzero(np.r_[True, sdst[1:] != sdst[:-1]])

    def conv(h, W, b):
        hs = (h * out_norm[:, None])[ssrc]
        sums = np.add.reduceat(hs, starts, axis=0)
        agg = np.zeros((n, h.shape[1]), f)
        agg[sdst[starts]] = sums
        return (agg * in_norm[:, None]) @ np.asarray(W, f) + np.asarray(b, f)

    h = np.maximum(np.asarray(x, f) @ np.asarray(w_lin, f)
                   + np.asarray(b_lin, f), 0.0)
    h = np.maximum(conv(h, w_c0, b_c0), 0.0)
    h = np.maximum(conv(h, w_c1, b_c1), 0.0)
    agent = h[np.arange(0, n, NPG)]
    return (agent @ np.asarray(w_emb, f) + np.asarray(b_emb, f)).astype(
        np.float32)


def assemble_out(core_outs, agent_rows):
    """Scatter per-core [8, EMB] outputs back to global agent row order."""
    full = np.empty((B, EMB), np.float32)
    full[agent_rows] = np.concatenate(core_outs, axis=0)
    return full


def kernel(x, src, dst, num_nodes, nodes_per_graph,
           w_lin, b_lin, w_c0, b_c0, w_c1, b_c1, w_emb, b_emb,
           _debug=None) -> np.ndarray:
    from concourse.bass_utils import run_bass_kernel_spmd

    assert int(num_nodes) == TOTAL and int(nodes_per_graph) == NPG
    if (np.any(np.asarray(b_lin)) or np.any(np.asarray(b_c0))
            or np.any(np.asarray(b_c1)) or np.any(np.asarray(b_emb))):
        src = np.asarray(src).astype(np.int64)
        dst = np.asarray(dst).astype(np.int64)
        return _kernel_numpy(np.asarray(x, np.float32), src, dst, w_lin,
                             b_lin, w_c0, b_c0, w_c1, b_c1, w_emb, b_emb)
    in_maps, cfg, agent_rows = make_in_maps(
        x, src, dst, w_lin, b_lin, w_c0, b_c0, w_c1, b_c1, w_emb, b_emb)

    nc = build_program(cfg)
    core_ids = list(range(NCORES))
    if _debug is not None:
        _debug["nc"] = nc
        _debug["in_maps"] = in_maps
        _debug["cfg"] = cfg
    res = run_bass_kernel_spmd(nc, in_maps, core_ids)
    return assemble_out([res.results[c]["out"] for c in range(NCORES)],
                        agent_rows)
